# revision 1
# baseline (speedup 1.0000x reference)
"""Trainium2 kernel for nn_DeepLinearTimeSeries.

The reference network is a 400-layer *linear* residual MLP: every step is
x <- x @ (W_i^T) [+ 0.1 * carry], with no nonlinearities anywhere. The whole
stack therefore collapses algebraically to a single matrix:

    out = x @ M_total,   M_total = T_enc @ T_temp @ T_dec @ W_out^T  (64 x 1)

where each block's transfer matrix is the product of its per-layer factors
(W_i^T + 0.1*I), with the first two layers of the encoder/temporal blocks
handled per the reference's carry pattern (T = W0^T W1^T + 0.1 I).

We fold the 400 64x64 factors on the host (trivial FLOPs, same f32
arithmetic regime as the reference), then run the remaining memory-bound
pass y = x @ m on 8 NeuronCores, data-parallel over the batch dim
(sharding_hint). Per core: x shard [32768, 64] f32 (8 MiB) -> y [32768, 1].

Device kernel (raw Bass, no Tile): the kernel is a DMA-paced stream. All
input DMAs are issued back-to-back on the sync-engine HW-DGE ring, which
executes them FIFO — so a single semaphore counter orders everything, and
chunk c is known complete at dma_sem >= 16*(c+2). The vector engine chases
the stream: per chunk, one broadcast multiply (y-vector m repeated via a
stride-0 AP) and one segmented window-64 reduce, each carrying exactly one
attached semaphore wait (TRN2 compute ISA structs have a single sync-wait
slot). Measured DVE chase rate (~4.6us/MiB) matches the per-core HBM
stream rate (~4.5us/MiB), so the kernel runs at the memory roofline.
Raw Bass avoids the Tile framework's entry barrier (~3.5us) and tail
drain/barrier structure (~5us).
"""

import numpy as np

import concourse.bass as bass
import concourse.mybir as mybir
from concourse.bass_utils import run_bass_kernel_spmd

# Problem constants (hardcoded per harness contract).
B, S, H = 128, 2048, 64
N_CORES = 8
RW = np.float32(0.1)
ROWS = B * S // N_CORES          # 32768 rows per core
P = 128                          # SBUF partitions
NPP = ROWS // P                  # 256 rows per partition
NCHUNK = 8
CHUNK = NPP // NCHUNK            # 32 row-groups per chunk
FREE = CHUNK * H                 # 2048 elements per partition per chunk
FP32 = mybir.dt.float32

# Extra kwargs for run_bass_kernel_spmd (test harness sets these for tracing).
RUN_KWARGS: dict = {}


def _collapse_weights(W_enc, W_temp, W_dec, W_out):
    """Fold the full linear stack into a single [H, 1] f32 matrix."""
    eye = np.eye(H, dtype=np.float32)

    def block_mat(Ws):
        # x1 = x0 W0^T ; x2 = x1 W1^T + 0.1 x0 ; then x <- x (Wi^T + 0.1 I)
        T = Ws[0].T @ Ws[1].T + RW * eye
        for Wi in Ws[2:]:
            T = T @ (Wi.T + RW * eye)
        return T

    M = block_mat(W_enc) @ block_mat(W_temp)
    for Wd in W_dec:
        M = M @ (Wd.T + RW * eye)
    return (M @ W_out.T).astype(np.float32)  # [H, 1]


def _build_bass():
    nc = bass.Bass()
    x = nc.dram_tensor("x", [ROWS, H], FP32, kind="ExternalInput")
    m = nc.dram_tensor("m", [P, H], FP32, kind="ExternalInput")
    y = nc.dram_tensor("y", [ROWS, 1], FP32, kind="ExternalOutput")

    x_t = x.rearrange("(p n) h -> p n h", p=P)        # [128, 256, 64]
    y_t = y.rearrange("(p n) one -> p (n one)", p=P)  # [128, 256]

    import contextlib

    with contextlib.ExitStack() as ctx:
        m_ld = ctx.enter_context(nc.sbuf_tensor("m_ld", [P, H], FP32))
        x_all = ctx.enter_context(
            nc.sbuf_tensor("x_all", [P, NCHUNK * FREE], FP32)
        )
        prod = ctx.enter_context(
            nc.sbuf_tensor("prod", [P, NCHUNK * FREE], FP32)
        )
        acc = ctx.enter_context(nc.sbuf_tensor("acc", [P, NPP], FP32))
        m_sem = ctx.enter_context(nc.semaphore("m_sem"))
        # DMA completions within one HWDGE queue are NOT ordered across
        # DMAs (packets spray over 16 SDMA engines), so each chunk gets
        # its own completion semaphore.
        c_sems = [
            ctx.enter_context(nc.semaphore(f"c_sem{i}")) for i in range(NCHUNK)
        ]
        y_sem = ctx.enter_context(nc.semaphore("y_sem"))
        dve_sem = ctx.enter_context(nc.semaphore("dve_sem"))
        block = ctx.enter_context(nc.Block())

        m_bc = m_ld[:].unsqueeze(1).broadcast_to((P, CHUNK, H))

        @block.sync
        def _(sync):
            sync.dma_start(m_ld[:], m[:]).then_inc(m_sem, 16)
            for c in range(NCHUNK):
                sync.dma_start(
                    x_all[:, c * FREE : (c + 1) * FREE].rearrange(
                        "p (n h) -> p n h", h=H
                    ),
                    x_t[:, c * CHUNK : (c + 1) * CHUNK, :],
                ).then_inc(c_sems[c], 16)
            # Output after the vector engine finishes the last reduce.
            sync.wait_ge(dve_sem, 2 * NCHUNK)
            sync.dma_start(y_t[:], acc[:]).then_inc(y_sem, 16)
            sync.wait_ge(y_sem, 16)

        @block.vector
        def _(vector):
            vector.wait_ge(m_sem, 16)
            for c in range(NCHUNK):
                lo = c * FREE
                vector.tensor_mul(
                    prod[:, lo : lo + FREE].rearrange("p (n h) -> p n h", h=H),
                    x_all[:, lo : lo + FREE].rearrange("p (n h) -> p n h", h=H),
                    m_bc,
                )._wait_ge(c_sems[c], 16).then_inc(dve_sem, 1)
                vector.tensor_reduce(
                    acc[:, c * CHUNK : (c + 1) * CHUNK],
                    prod[:, lo : lo + FREE].rearrange("p (n h) -> p n h", h=H),
                    axis=mybir.AxisListType.X,
                    op=mybir.AluOpType.add,
                )._wait_ge(dve_sem, 2 * c + 1).then_inc(dve_sem, 1)

    return nc


def kernel(**inputs: np.ndarray) -> np.ndarray:
    x = np.asarray(inputs["x"], dtype=np.float32)
    m = _collapse_weights(
        np.asarray(inputs["W_enc"], dtype=np.float32),
        np.asarray(inputs["W_temp"], dtype=np.float32),
        np.asarray(inputs["W_dec"], dtype=np.float32),
        np.asarray(inputs["W_out"], dtype=np.float32),
    )
    m_bcast = np.ascontiguousarray(np.broadcast_to(m.reshape(1, H), (P, H)))

    nc = _build_bass()
    shard_b = B // N_CORES
    in_maps = [
        {
            "x": np.ascontiguousarray(
                x[i * shard_b : (i + 1) * shard_b].reshape(ROWS, H)
            ),
            "m": m_bcast,
        }
        for i in range(N_CORES)
    ]
    res = run_bass_kernel_spmd(
        nc, in_maps, core_ids=list(range(N_CORES)), **RUN_KWARGS
    )
    return np.concatenate(
        [r["y"].reshape(shard_b, S, 1) for r in res.results], axis=0
    )



# revision 4
# speedup vs baseline: 1.7080x; 1.7080x over previous
"""Trainium2 kernel for nn_DeepLinearTimeSeries.

The reference network is a 400-layer *linear* residual MLP: every step is
x <- x @ (W_i^T) [+ 0.1 * carry], with no nonlinearities anywhere. The whole
stack therefore collapses algebraically to a single matrix:

    out = x @ M_total,   M_total = T_enc @ T_temp @ T_dec @ W_out^T  (64 x 1)

We fold the 400 64x64 factors on the host (trivial FLOPs), then run the
remaining memory-bound pass y = x @ m on 8 NeuronCores, data-parallel over
the batch dim (sharding_hint).

Device kernel v2 (PE matvec, fp16 stream):
  * x is packed on the host to fp16 [128, 16384] per core: column n holds
    rows (2n, 2n+1) interleaved over h -- k = parity*64 + h. This halves the
    HBM stream (4 MiB/core vs 8 MiB f32) and puts the contraction dim on
    SBUF partitions so the TensorEngine can do the multiply+reduce:
        y[2n+p] = sum_k lhsT[k, p] * xpack[k, n]
    with lhsT = [[m;0] , [0;m]] zero-padded to 32 columns.
  * 32 matmuls of N=512 write PSUM regions [32a:32a+32, bank b] with
    a = i%4 (column-group position in the PE array) and b = i//4, so each
    PSUM bank is fully written across all 128 partitions. A [128, 1024]
    PSUM->SBUF copy per chunk (DVE for even chunks, ACT for odd) is then
    cheap (~1us) because all partition lanes are engaged; only partitions
    {32a, 32a+1} carry real data (rest are zero columns of lhsT).
  * 8 warmup matmuls on garbage data at kernel start push the PE past the
    HAM 3.4us activity window so the real matmuls run at 2.4 GHz.
  * DVE is off the critical path entirely (the baseline's mul+reduce chain
    was 36us of DVE time; PE does the same work in ~7us hidden under DMA).
  * y leaves via 4 small DMAs [2, 4096] f32 (one per column-group), host
    un-permutes.
"""

import contextlib

import numpy as np

import concourse.bass as bass
import concourse.mybir as mybir
from concourse.bass_utils import run_bass_kernel_spmd

# Problem constants (hardcoded per harness contract).
B, S, H = 128, 2048, 64
N_CORES = 8
RW = np.float32(0.1)
ROWS = B * S // N_CORES          # 32768 rows per core
P = 128                          # SBUF partitions = 2 parities x 64 h
NCOL = ROWS // 2                 # 16384 packed moving columns per core
NCHUNK = 4
CCOL = NCOL // NCHUNK            # 4096 columns per DMA chunk (1 MiB fp16)
MM_N = 512                       # moving free dim per matmul (1 PSUM bank)
MM_PER_CHUNK = CCOL // MM_N      # 8
N_MM = NCHUNK * MM_PER_CHUNK     # 32
M_PAD = 32                       # stationary cols: 2 used + 30 zero (col-grp)
N_WARM = 8                       # HAM warmup matmuls
FP16 = mybir.dt.float16
FP32 = mybir.dt.float32

# Extra kwargs for run_bass_kernel_spmd (test harness sets these for tracing).
RUN_KWARGS: dict = {}


def _collapse_weights(W_enc, W_temp, W_dec, W_out):
    """Fold the full linear stack into a single [H] f32 vector."""
    eye = np.eye(H, dtype=np.float32)

    def block_mat(Ws):
        # x1 = x0 W0^T ; x2 = x1 W1^T + 0.1 x0 ; then x <- x (Wi^T + 0.1 I)
        T = Ws[0].T @ Ws[1].T + RW * eye
        for Wi in Ws[2:]:
            T = T @ (Wi.T + RW * eye)
        return T

    M = block_mat(W_enc) @ block_mat(W_temp)
    for Wd in W_dec:
        M = M @ (Wd.T + RW * eye)
    return (M @ W_out.T).astype(np.float32).reshape(H)  # [H]


def _build_bass():
    nc = bass.Bass()
    xp = nc.dram_tensor("xp", [P, NCOL], FP16, kind="ExternalInput")
    w = nc.dram_tensor("w", [P, M_PAD], FP16, kind="ExternalInput")
    y = nc.dram_tensor("y", [8, NCOL // NCHUNK], FP32, kind="ExternalOutput")

    with contextlib.ExitStack() as ctx:
        w_sb = ctx.enter_context(nc.sbuf_tensor("w_sb", [P, M_PAD], FP16))
        x_sb = ctx.enter_context(nc.sbuf_tensor("x_sb", [P, NCOL], FP16))
        y_sb = ctx.enter_context(nc.sbuf_tensor("y_sb", [P, N_MM * MM_N // 4], FP32))
        ps = ctx.enter_context(nc.psum_tensor("ps", [P, 4096], FP32))
        # DMA completions within one HWDGE queue are NOT ordered across
        # DMAs, so each chunk gets its own completion semaphore; the weight
        # DMA shares chunk 0's semaphore additively (wait >= 32).
        c_sems = [
            ctx.enter_context(nc.semaphore(f"c_sem{i}")) for i in range(NCHUNK)
        ]
        pe_sem = ctx.enter_context(nc.semaphore("pe_sem"))
        cp_sem = ctx.enter_context(nc.semaphore("cp_sem"))
        y_sem = ctx.enter_context(nc.semaphore("y_sem"))
        block = ctx.enter_context(nc.Block())

        @block.sync
        def _(sync):
            sync.dma_start(w_sb[:], w[:]).then_inc(c_sems[0], 16)
            for c in range(NCHUNK):
                sync.dma_start(
                    x_sb[:, c * CCOL : (c + 1) * CCOL],
                    xp[:, c * CCOL : (c + 1) * CCOL],
                ).then_inc(c_sems[c], 16)
            # Output after both copy engines finish evacuating PSUM.
            sync.wait_ge(cp_sem, NCHUNK)
            for a in (0, 1):
                sync.dma_start(
                    y[2 * a : 2 * a + 2, :], y_sb[32 * a : 32 * a + 2, :]
                ).then_inc(y_sem, 16)
            sync.wait_ge(y_sem, 64)

        @block.tensor
        def _(tensor):
            # HAM warmup: ~3.4us of cold matmuls on garbage SBUF so the PE
            # clock-gate opens before the first real chunk lands. Regions are
            # overwritten by the real matmuls (start=True resets PSUM).
            for k in range(N_WARM):
                src = (N_MM - N_WARM + k) * MM_N
                tensor.matmul(
                    ps[0:M_PAD, k * MM_N : (k + 1) * MM_N],
                    w_sb[:],
                    x_sb[:, src : src + MM_N],
                    start=True,
                    stop=True,
                )
            for c in range(NCHUNK):
                for j in range(MM_PER_CHUNK):
                    i = c * MM_PER_CHUNK + j
                    a, b = i % 4, i // 4
                    mm = tensor.matmul(
                        ps[32 * a : 32 * a + M_PAD, b * MM_N : (b + 1) * MM_N],
                        w_sb[:],
                        x_sb[:, i * MM_N : (i + 1) * MM_N],
                        start=True,
                        stop=True,
                        tile_position=(0, 32 * a),
                    )
                    if j == 0:
                        mm._wait_ge(c_sems[c], 32 if c == 0 else 16)
                    if j == MM_PER_CHUNK - 1:
                        mm.then_inc(pe_sem, 1)

        @block.vector
        def _(vector):
            for c in (0, 2):
                lo = c * 2 * MM_N
                vector.tensor_copy(
                    y_sb[:, lo : lo + 2 * MM_N], ps[:, lo : lo + 2 * MM_N]
                )._wait_ge(pe_sem, c + 1).then_inc(cp_sem, 1)

        @block.scalar
        def _(scalar):
            for c in (1, 3):
                lo = c * 2 * MM_N
                scalar.copy(
                    y_sb[:, lo : lo + 2 * MM_N], ps[:, lo : lo + 2 * MM_N]
                )._wait_ge(pe_sem, c + 1).then_inc(cp_sem, 1)
            scalar.wait_ge(cp_sem, NCHUNK)
            for a in (2, 3):
                scalar.dma_start(
                    y[2 * a : 2 * a + 2, :], y_sb[32 * a : 32 * a + 2, :]
                ).then_inc(y_sem, 16)

    return nc


def kernel(**inputs: np.ndarray) -> np.ndarray:
    x = np.asarray(inputs["x"], dtype=np.float32)
    m = _collapse_weights(
        np.asarray(inputs["W_enc"], dtype=np.float32),
        np.asarray(inputs["W_temp"], dtype=np.float32),
        np.asarray(inputs["W_dec"], dtype=np.float32),
        np.asarray(inputs["W_out"], dtype=np.float32),
    )
    w_np = np.zeros((P, M_PAD), dtype=np.float16)
    w_np[0:H, 0] = m.astype(np.float16)
    w_np[H : 2 * H, 1] = m.astype(np.float16)

    # Pack x per core: [core, k = parity*64 + h, n] in fp16.
    x16 = x.astype(np.float16)
    xp_all = np.ascontiguousarray(
        x16.reshape(N_CORES, NCOL, 2, H).transpose(0, 2, 3, 1)
    ).reshape(N_CORES, P, NCOL)

    nc = _build_bass()
    in_maps = [{"xp": xp_all[i], "w": w_np} for i in range(N_CORES)]
    res = run_bass_kernel_spmd(
        nc, in_maps, core_ids=list(range(N_CORES)), **RUN_KWARGS
    )

    # Un-permute: y_dev[2a+m, 512b+j] = y[1024*(4b+a) + 2j + m].
    shard_b = B // N_CORES
    outs = []
    for r in res.results:
        yd = r["y"].reshape(4, 2, NCHUNK * 2, MM_N)
        y_core = np.ascontiguousarray(yd.transpose(2, 0, 3, 1)).reshape(ROWS)
        outs.append(y_core.reshape(shard_b, S, 1))
    return np.concatenate(outs, axis=0).astype(np.float32)


# revision 5
# speedup vs baseline: 1.8742x; 1.0973x over previous
"""Trainium2 kernel for nn_DeepLinearTimeSeries.

The reference network is a 400-layer *linear* residual MLP: every step is
x <- x @ (W_i^T) [+ 0.1 * carry], with no nonlinearities anywhere. The whole
stack therefore collapses algebraically to a single matrix:

    out = x @ M_total,   M_total = T_enc @ T_temp @ T_dec @ W_out^T  (64 x 1)

We fold the 400 64x64 factors on the host (trivial FLOPs), then run the
remaining memory-bound pass y = x @ m on 8 NeuronCores, data-parallel over
the batch dim (sharding_hint).

Device kernel v3 (PE matvec, fp16 stream):
  * x is packed on the host to fp16 [128, 16384] per core: column n holds
    rows (2n, 2n+1) interleaved over h -- k = parity*64 + h. This halves the
    HBM stream (4 MiB/core) and puts the contraction dim on SBUF partitions
    so the TensorEngine does the multiply+reduce:
        y[2n+p] = sum_k lhsT[k, p] * xpack[k, n]
  * lhsT [128, 32] holds the (even, odd) m-pair in columns (0,1) AND a
    duplicate in columns (4,5). 32 matmuls of N=512 go to PSUM regions
    [32a : 32a+32, bank b] with column-group a = i%4, bank b = i//4, so
    every PSUM bank is written across all 128 partitions and the 4
    column-group tiles execute CONCURRENTLY in the PE array (~4ns stagger).
    Useful rows per group alternate between the (0,1) and (4,5) pair so the
    4 output slices {0,1},{36,37},{64,65},{100,101} map to 4 *distinct*
    SDMA engines (pairs p and p+32 share an engine).
  * PSUM evacuation: per chunk, DVE copies the left 512 columns and ACT the
    right 512, casting f32 -> fp16 (halves the SBUF write traffic that
    competes with the input DMA stream for the 435 GB/s fabric).
  * y leaves as fp16 in 2 waves of 4 DMAs (wave 0 after chunks 0-1 are
    evacuated, hidden under the stream; wave 1 is the only tail). Host
    un-permutes and upcasts.
  * 6 warmup matmuls on garbage data at kernel start open the PE HAM clock
    gate before the real matmuls; the column-group concurrency gives PE a
    4x margin over the stream even when cold.
"""

import contextlib

import numpy as np

import concourse.bass as bass
import concourse.mybir as mybir
from concourse.bass_utils import run_bass_kernel_spmd

# Problem constants (hardcoded per harness contract).
B, S, H = 128, 2048, 64
N_CORES = 8
RW = np.float32(0.1)
ROWS = B * S // N_CORES          # 32768 rows per core
P = 128                          # SBUF partitions = 2 parities x 64 h
NCOL = ROWS // 2                 # 16384 packed moving columns per core
NCHUNK = 4
CCOL = NCOL // NCHUNK            # 4096 columns per DMA chunk (1 MiB fp16)
MM_N = 512                       # moving free dim per matmul (1 PSUM bank)
MM_PER_CHUNK = CCOL // MM_N      # 8
N_MM = NCHUNK * MM_PER_CHUNK     # 32
M_PAD = 32                       # stationary cols: pairs at (0,1),(4,5) + 0s
N_WARM = 6                       # HAM warmup matmuls
YCOL = NCOL // NCHUNK            # 4096 y columns per output row-pair
# Output slice base partitions: column-group a uses pair (0,1) for even a
# and (4,5) for odd a, landing the 4 slices on 4 distinct SDMA engines.
Y_BASE = [0, 36, 64, 100]
FP16 = mybir.dt.float16
FP32 = mybir.dt.float32

# Extra kwargs for run_bass_kernel_spmd (test harness sets these for tracing).
RUN_KWARGS: dict = {}


def _collapse_weights(W_enc, W_temp, W_dec, W_out):
    """Fold the full linear stack into a single [H] f32 vector."""
    eye = np.eye(H, dtype=np.float32)

    def block_mat(Ws):
        # x1 = x0 W0^T ; x2 = x1 W1^T + 0.1 x0 ; then x <- x (Wi^T + 0.1 I)
        T = Ws[0].T @ Ws[1].T + RW * eye
        for Wi in Ws[2:]:
            T = T @ (Wi.T + RW * eye)
        return T

    M = block_mat(W_enc) @ block_mat(W_temp)
    for Wd in W_dec:
        M = M @ (Wd.T + RW * eye)
    return (M @ W_out.T).astype(np.float32).reshape(H)  # [H]


def _build_bass():
    nc = bass.Bass()
    xp = nc.dram_tensor("xp", [P, NCOL], FP16, kind="ExternalInput")
    w = nc.dram_tensor("w", [P, M_PAD], FP16, kind="ExternalInput")
    y = nc.dram_tensor("y", [8, YCOL], FP16, kind="ExternalOutput")

    with contextlib.ExitStack() as ctx:
        w_sb = ctx.enter_context(nc.sbuf_tensor("w_sb", [P, M_PAD], FP16))
        x_sb = ctx.enter_context(nc.sbuf_tensor("x_sb", [P, NCOL], FP16))
        y_sb = ctx.enter_context(nc.sbuf_tensor("y_sb", [P, 4096], FP16))
        ps = ctx.enter_context(nc.psum_tensor("ps", [P, 4096], FP32))
        # DMA completions within one HWDGE queue are NOT ordered across
        # DMAs, so each chunk gets its own completion semaphore; the weight
        # DMA shares chunk 0's semaphore additively (wait >= 32).
        c_sems = [
            ctx.enter_context(nc.semaphore(f"c_sem{i}")) for i in range(NCHUNK)
        ]
        pe_sem = ctx.enter_context(nc.semaphore("pe_sem"))
        cpA = ctx.enter_context(nc.semaphore("cpA"))  # DVE copy halves
        cpB = ctx.enter_context(nc.semaphore("cpB"))  # ACT copy halves
        y_sem = ctx.enter_context(nc.semaphore("y_sem"))
        block = ctx.enter_context(nc.Block())

        def y_out_wave(eng, groups, half):
            lo = half * 2048
            for a in groups:
                eng.dma_start(
                    y[2 * a : 2 * a + 2, lo : lo + 2048],
                    y_sb[Y_BASE[a] : Y_BASE[a] + 2, lo : lo + 2048],
                ).then_inc(y_sem, 16)

        @block.sync
        def _(sync):
            for c in range(NCHUNK):
                sync.dma_start(
                    x_sb[:, c * CCOL : (c + 1) * CCOL],
                    xp[:, c * CCOL : (c + 1) * CCOL],
                ).then_inc(c_sems[c], 16)
            sync.wait_ge(cpA, 2)
            sync.wait_ge(cpB, 2)
            y_out_wave(sync, (0, 1), 0)
            sync.wait_ge(cpA, 4)
            sync.wait_ge(cpB, 4)
            y_out_wave(sync, (0, 1), 1)
            sync.wait_ge(y_sem, 128)

        @block.tensor
        def _(tensor):
            # HAM warmup: cold matmuls on garbage SBUF so the PE clock-gate
            # opens before the real work. Regions are overwritten by the real
            # matmuls (start=True resets PSUM).
            for k in range(N_WARM):
                src = (N_MM - N_WARM + k) * MM_N
                tensor.matmul(
                    ps[0:M_PAD, k * MM_N : (k + 1) * MM_N],
                    w_sb[:],
                    x_sb[:, src : src + MM_N],
                    start=True,
                    stop=True,
                )
            for c in range(NCHUNK):
                for j in range(MM_PER_CHUNK):
                    i = c * MM_PER_CHUNK + j
                    a, b = i % 4, i // 4
                    mm = tensor.matmul(
                        ps[32 * a : 32 * a + M_PAD, b * MM_N : (b + 1) * MM_N],
                        w_sb[:],
                        x_sb[:, i * MM_N : (i + 1) * MM_N],
                        start=True,
                        stop=True,
                        tile_position=(0, 32 * a),
                    )
                    if j == 0:
                        mm._wait_ge(c_sems[c], 32 if c == 0 else 16)
                    if j == MM_PER_CHUNK - 1:
                        mm.then_inc(pe_sem, 1)

        @block.vector
        def _(vector):
            # Left 512-column half of each chunk's PSUM region, f32 -> fp16.
            for c in range(NCHUNK):
                lo = c * 2 * MM_N
                vector.tensor_copy(
                    y_sb[:, lo : lo + MM_N], ps[:, lo : lo + MM_N]
                )._wait_ge(pe_sem, c + 1).then_inc(cpA, 1)

        @block.scalar
        def _(scalar):
            scalar.dma_start(w_sb[:], w[:]).then_inc(c_sems[0], 16)
            # Right 512-column half of each chunk's PSUM region.
            for c in range(NCHUNK):
                lo = c * 2 * MM_N + MM_N
                scalar.copy(
                    y_sb[:, lo : lo + MM_N], ps[:, lo : lo + MM_N]
                )._wait_ge(pe_sem, c + 1).then_inc(cpB, 1)
                if c == 1:
                    scalar.wait_ge(cpA, 2)
                    y_out_wave(scalar, (2, 3), 0)
            scalar.wait_ge(cpA, 4)
            y_out_wave(scalar, (2, 3), 1)

    return nc


def kernel(**inputs: np.ndarray) -> np.ndarray:
    x = np.asarray(inputs["x"], dtype=np.float32)
    m = _collapse_weights(
        np.asarray(inputs["W_enc"], dtype=np.float32),
        np.asarray(inputs["W_temp"], dtype=np.float32),
        np.asarray(inputs["W_dec"], dtype=np.float32),
        np.asarray(inputs["W_out"], dtype=np.float32),
    )
    m16 = m.astype(np.float16)
    w_np = np.zeros((P, M_PAD), dtype=np.float16)
    for col0 in (0, 4):
        w_np[0:H, col0] = m16
        w_np[H : 2 * H, col0 + 1] = m16

    # Pack x per core: [core, k = parity*64 + h, n] in fp16.
    x16 = x.astype(np.float16)
    xp_all = np.ascontiguousarray(
        x16.reshape(N_CORES, NCOL, 2, H).transpose(0, 2, 3, 1)
    ).reshape(N_CORES, P, NCOL)

    nc = _build_bass()
    in_maps = [{"xp": xp_all[i], "w": w_np} for i in range(N_CORES)]
    res = run_bass_kernel_spmd(
        nc, in_maps, core_ids=list(range(N_CORES)), **RUN_KWARGS
    )

    # Un-permute: y_dev[2a+m, 512b+j] = y[1024*(4b+a) + 2j + m].
    shard_b = B // N_CORES
    outs = []
    for r in res.results:
        yd = r["y"].astype(np.float32).reshape(4, 2, NCHUNK * 2, MM_N)
        y_core = np.ascontiguousarray(yd.transpose(2, 0, 3, 1)).reshape(ROWS)
        outs.append(y_core.reshape(shard_b, S, 1))
    return np.concatenate(outs, axis=0).astype(np.float32)
